# revision 44
# baseline (speedup 1.0000x reference)
"""Trainium2 Bass kernel for nn_Attention (GroupNorm -> linear attention ->
out_proj -> GroupNorm -> gated residual).

Sharding: data-parallel over batch B=8 across the 8 NeuronCores (one batch
element per core, no collectives).

Per-core pipeline (hidden = x [F=512, S=8192] f32):
  A) stream hidden from DRAM; cast to bf16 slab in SBUF; bn_stats per channel
  B) finalize GroupNorm1 (group=16 channels) scale/bias per channel via tiny
     selector matmuls; normalize the bf16 slab in place (folds GN1)
  C) K/V projection in transposed layout out[s,m] = x_chunk^T @ Wkv;
     k = elu(.)+1 = exp(min(.,0)) + max(.,0); accumulate KV[h,dk,dv] and
     ksum[h,dk] in PSUM across all of S via per-head matmuls
  D) Q projection out[m,s] = Wq^T @ x; elu+1; z = ksum . q per (head, s);
     attn = blockdiag(KV)^T @ q; eviction fused with *1/z (1/z broadcast
     across partitions via a bf16 hi/lo selector matmul pair);
     y_pre = out_proj^T @ attn; GN2 stats on the fly (sum via matmul trick,
     sumsq via ACT Square accum); y_pre -> bf16 slab
  E) finalize GN2 scale/bias, fold the rezero gate
  F) out = hidden + gate*(gn2(y_pre)): stream hidden again, add, store
"""

import numpy as np
import ml_dtypes
from contextlib import ExitStack

import concourse.bass as bass
import concourse.bacc as bacc
import concourse.tile as tile
import concourse.mybir as mybir
from concourse.bass_utils import run_bass_kernel_spmd

F32 = mybir.dt.float32
BF16 = mybir.dt.bfloat16
AF = mybir.ActivationFunctionType
OP = mybir.AluOpType

B, F, S, H = 8, 512, 8192, 8
D = F // H            # 64
GROUPS = F // 16      # 32
EPS = 1e-8
P = 128               # partitions
FB = F // P           # 4 f-blocks
ST = 512              # s-tile (free dim per tile)
NT = S // ST          # 16 s-tiles
SC = 128              # s-chunk for transposed kv matmuls
NSC = S // SC         # 64 s-chunks
MB = F // P           # 4 m-chunks (q rows / attn rows)
KVM = 2 * F           # 1024 columns of the kv projection (k then v)

N_CORES = 8


def _build_program(has_q_bias: bool, has_kv_bias: bool, dbg: bool = False,
                   upto: str = "F", dsub: int = 7):
    rank = {"A": 0, "C": 1, "D": 2, "E": 3, "F": 4}[upto]
    nc = bacc.Bacc(trn_type="TRN2", target_bir_lowering=False, debug=False,
                   num_devices=N_CORES)

    hidden = nc.dram_tensor("hidden", [F, S], F32, kind="ExternalInput").ap()
    wq = nc.dram_tensor("wq_t", [F, F], BF16, kind="ExternalInput").ap()
    wkv = nc.dram_tensor("wkv_t", [F, KVM], BF16, kind="ExternalInput").ap()
    pt = nc.dram_tensor("p_t", [F, F], BF16, kind="ExternalInput").ap()
    selg = nc.dram_tensor("sel_g", [P, 8], F32, kind="ExternalInput").ap()
    selb = nc.dram_tensor("sel_b", [8, P], F32, kind="ExternalInput").ap()
    selzb = nc.dram_tensor("sel_zb", [8, MB * P], BF16, kind="ExternalInput").ap()
    g1 = nc.dram_tensor("gamma1c", [P, FB], F32, kind="ExternalInput").ap()
    b1 = nc.dram_tensor("beta1c", [P, FB], F32, kind="ExternalInput").ap()
    g2 = nc.dram_tensor("gamma2c", [P, FB], F32, kind="ExternalInput").ap()
    b2 = nc.dram_tensor("beta2c", [P, FB], F32, kind="ExternalInput").ap()
    gate = nc.dram_tensor("gatec", [P, FB], F32, kind="ExternalInput").ap()
    bq = bkv = None
    if has_q_bias:
        bq = nc.dram_tensor("bq", [P, MB], F32, kind="ExternalInput").ap()
    if has_kv_bias:
        bkv = nc.dram_tensor("bkv", [1, KVM], BF16, kind="ExternalInput").ap()
    out = nc.dram_tensor("out", [F, S], F32, kind="ExternalOutput").ap()
    if dbg:
        dbg_xn = nc.dram_tensor("dbg_xn", [P, FB, S], BF16,
                                kind="ExternalOutput").ap()
        dbg_y = nc.dram_tensor("dbg_y", [P, FB, S], BF16,
                               kind="ExternalOutput").ap()
        dbg_kv2 = nc.dram_tensor("dbg_kv2", [P, MB, P], BF16,
                                 kind="ExternalOutput").ap()
        dbg_ks2 = nc.dram_tensor("dbg_ks2", [P, MB, 8], BF16,
                                 kind="ExternalOutput").ap()
        dbg_zr = nc.dram_tensor("dbg_zr", [NT, 8, ST], F32,
                                kind="ExternalOutput").ap()
        dbg_sb2 = nc.dram_tensor("dbg_sb2", [P, FB, 4], F32,
                                 kind="ExternalOutput").ap()

    # channel-major views: [c, s] -> [p, fb, s] with c = fb*128 + p
    hidden_v = hidden.rearrange("(fb p) s -> p fb s", p=P)
    out_v = out.rearrange("(fb p) s -> p fb s", p=P)
    wq_v = wq.rearrange("(fb p) m -> p fb m", p=P)
    wkv_v = wkv.rearrange("(fb p) m -> p fb m", p=P)
    pt_v = pt.rearrange("(mc p) f -> p mc f", p=P)

    with tile.TileContext(nc) as tc, ExitStack() as ctx:
        const = ctx.enter_context(tc.tile_pool(name="const", bufs=1))
        slab = ctx.enter_context(tc.tile_pool(name="slab", bufs=1))
        stats = ctx.enter_context(tc.tile_pool(name="stats", bufs=1))
        small = ctx.enter_context(tc.tile_pool(name="small", bufs=2))

        # ---- constants / weights in SBUF ----
        wq_sb = const.tile([P, FB, F], BF16)
        nc.sync.dma_start(wq_sb[:], wq_v)
        wkv_sb = const.tile([P, FB, KVM], BF16)
        nc.sync.dma_start(wkv_sb[:], wkv_v)
        pt_sb = const.tile([P, MB, F], BF16)
        nc.sync.dma_start(pt_sb[:], pt_v)
        selg_sb = const.tile([P, 8], F32)
        nc.sync.dma_start(selg_sb[:], selg)
        selb_sb = const.tile([8, P], F32)
        nc.sync.dma_start(selb_sb[:], selb)
        selzb_sb = const.tile([8, MB * P], BF16)
        nc.sync.dma_start(selzb_sb[:], selzb)
        g1_sb = const.tile([P, FB], F32)
        nc.sync.dma_start(g1_sb[:], g1)
        b1_sb = const.tile([P, FB], F32)
        nc.sync.dma_start(b1_sb[:], b1)
        g2_sb = const.tile([P, FB], F32)
        nc.sync.dma_start(g2_sb[:], g2)
        b2_sb = const.tile([P, FB], F32)
        nc.sync.dma_start(b2_sb[:], b2)
        gate_sb = const.tile([P, FB], F32)
        nc.sync.dma_start(gate_sb[:], gate)
        if has_q_bias:
            bq_sb = const.tile([P, MB], F32)
            nc.sync.dma_start(bq_sb[:], bq)
        if has_kv_bias:
            bkv_sb = const.tile([1, KVM], BF16)
            nc.sync.dma_start(bkv_sb[:], bkv)
            ones_row = const.tile([1, P], BF16)
            nc.vector.memset(ones_row[:], 1.0)
        ones_col = const.tile([P, 1], BF16)
        nc.vector.memset(ones_col[:], 1.0)

        x_slab = slab.tile([P, FB, S], BF16)   # raw hidden, then normalized
        y_slab = slab.tile([P, FB, S], BF16)   # pre-GN2 branch output

        # =========== Phase A: load + cast + GN1 stats ===========
        bnout = stats.tile([P, FB, NT, 6], F32)
        with tc.tile_pool(name="astage", bufs=3) as astage:
            for t in range(NT):
                st = astage.tile([P, FB, ST], F32)
                nc.gpsimd.dma_start(st[:], hidden_v[:, :, t * ST:(t + 1) * ST])
                nc.gpsimd.tensor_copy(x_slab[:, :, t * ST:(t + 1) * ST], st[:])
                for fb in range(FB):
                    nc.vector.bn_stats(bnout[:, fb, t, :],
                                       x_slab[:, fb, t * ST:(t + 1) * ST])

        # =========== Phase B: finalize GN1, normalize slab ===========
        def groupnorm_finalize(mean_c, e2_c, gamma_sb, beta_sb, pool, ppool):
            """mean_c, e2_c: [P, FB] f32 per-channel mean and E[x^2].
            Returns (scale, bias) [P, FB] f32 per channel, with group stats
            (16 consecutive channels per group) folded in."""
            cs = pool.tile([P, 8], F32, tag="gn_cs")
            nc.vector.tensor_copy(cs[:, 0:FB], mean_c)
            nc.vector.tensor_copy(cs[:, FB:8], e2_c)
            gsum_ps = ppool.tile([8, 8], F32, tag="ps_small")
            nc.tensor.matmul(gsum_ps[:], selg_sb[:], cs[:], start=True, stop=True)
            gsum = pool.tile([8, 8], F32, tag="gn_gsum")
            nc.vector.tensor_copy(gsum[:], gsum_ps[:])
            bc_ps = ppool.tile([P, 8], F32, tag="ps_small")
            nc.tensor.matmul(bc_ps[:], selb_sb[:], gsum[:], start=True, stop=True)
            mug = pool.tile([P, FB], F32, tag="gn_mug")
            nc.vector.tensor_scalar_mul(mug[:], bc_ps[:, 0:FB], 1.0 / 16.0)
            varg = pool.tile([P, FB], F32, tag="gn_varg")
            nc.vector.tensor_scalar_mul(varg[:], bc_ps[:, FB:8], 1.0 / 16.0)
            t1 = pool.tile([P, FB], F32, tag="gn_t1")
            nc.vector.tensor_tensor(t1[:], mug[:], mug[:], op=OP.mult)
            nc.vector.tensor_tensor(varg[:], varg[:], t1[:], op=OP.subtract)
            nc.vector.tensor_scalar_add(varg[:], varg[:], EPS)
            stdg = pool.tile([P, FB], F32, tag="gn_stdg")
            nc.scalar.activation(stdg[:], varg[:], AF.Sqrt)
            rstd = pool.tile([P, FB], F32, tag="gn_rstd")
            scr = pool.tile([P, FB], F32, tag="gn_scr")
            nc.vector.reciprocal_approx_accurate(out=rstd[:], in_=stdg[:],
                                                 scratch=scr[:])
            scale = pool.tile([P, FB], F32, tag="gn_scale")
            nc.vector.tensor_tensor(scale[:], gamma_sb, rstd[:], op=OP.mult)
            t2 = pool.tile([P, FB], F32, tag="gn_t2")
            nc.vector.tensor_tensor(t2[:], mug[:], scale[:], op=OP.mult)
            bias = pool.tile([P, FB], F32, tag="gn_bias")
            nc.vector.tensor_tensor(bias[:], beta_sb, t2[:], op=OP.subtract)
            return scale, bias

        aggr = stats.tile([P, FB, 2], F32)
        for fb in range(FB):
            nc.vector.bn_aggr(aggr[:, fb, :], bnout[:, fb, :, :])
        mean_c = stats.tile([P, FB], F32)
        nc.vector.tensor_copy(mean_c[:], aggr[:, :, 0])
        e2_c = stats.tile([P, FB], F32)
        nc.vector.tensor_tensor(e2_c[:], aggr[:, :, 0], aggr[:, :, 0], op=OP.mult)
        nc.vector.tensor_tensor(e2_c[:], e2_c[:], aggr[:, :, 1], op=OP.add)
        with tc.tile_pool(name="psB", bufs=2, space="PSUM") as psB:
            scale1, bias1 = groupnorm_finalize(mean_c[:], e2_c[:], g1_sb[:],
                                               b1_sb[:], small, psB)
        for fb in range(FB):
            nc.gpsimd.tensor_scalar(out=x_slab[:, fb, :], in0=x_slab[:, fb, :],
                                    scalar1=scale1[:, fb:fb + 1],
                                    scalar2=bias1[:, fb:fb + 1],
                                    op0=OP.mult, op1=OP.add)

        if rank < 4:
            # truncated build (bisection): dump zeros to out
            with tc.tile_pool(name="eo", bufs=1) as eo:
                zt = eo.tile([P, FB, ST], F32)
                nc.vector.memset(zt[:], 0.0)
                for t in range(NT):
                    nc.gpsimd.dma_start(out_v[:, :, t * ST:(t + 1) * ST], zt[:])

        # =========== Phase C: K/V projection + KV/ksum accumulation ===========
        kv2_sb = const.tile([P, MB, P], BF16)     # blockdiag KV per m-chunk
        ksum2_sb = const.tile([P, MB, 8], BF16)   # z-matmul lhsT per m-chunk
        if rank >= 1:
            with tc.tile_pool(name="ckv", bufs=2, space="PSUM") as ckv_pool, \
                 tc.tile_pool(name="kvacc", bufs=1, space="PSUM") as kvacc_pool, \
                 tc.tile_pool(name="celu", bufs=3) as celu:
                kv_acc = kvacc_pool.tile([P, 256], F32)  # KV[h]:[h%2*64, h//2*64]
                ks_acc = kvacc_pool.tile([P, MB], F32)   # ksum[h]:[h%2*64, h//2]
                nc.vector.memset(kv_acc[:], 0.0)
                nc.vector.memset(ks_acc[:], 0.0)
                for sc in range(NSC):
                    kvp = ckv_pool.tile([P, 2, ST], F32)
                    first = True
                    if has_kv_bias:
                        for j in range(2):
                            nc.tensor.matmul(kvp[:, j, :], ones_row[:],
                                             bkv_sb[:, j * ST:(j + 1) * ST],
                                             start=True, stop=False)
                        first = False
                    for fb in range(FB):
                        xc = x_slab[:, fb, sc * SC:(sc + 1) * SC]
                        for j in range(2):
                            nc.tensor.matmul(kvp[:, j, :], xc,
                                             wkv_sb[:, fb, j * ST:(j + 1) * ST],
                                             start=(first and fb == 0),
                                             stop=(fb == FB - 1))
                    # k = exp(min(klin,0)) + max(klin,0); v = cast
                    tmin = celu.tile([P, ST], BF16, tag="tmin")
                    nc.vector.tensor_scalar_min(tmin[:], kvp[:, 0, :], 0.0)
                    e = celu.tile([P, ST], BF16, tag="e")
                    nc.scalar.activation(e[:], tmin[:], AF.Exp)
                    k = celu.tile([P, ST], BF16, tag="k")
                    nc.vector.scalar_tensor_tensor(out=k[:], in0=kvp[:, 0, :],
                                                   scalar=0.0, in1=e[:],
                                                   op0=OP.max, op1=OP.add)
                    v = celu.tile([P, ST], BF16, tag="v")
                    nc.scalar.activation(v[:], kvp[:, 1, :], AF.Copy)
                    for h in range(H):
                        kh = k[:, h * D:(h + 1) * D]
                        vh = v[:, h * D:(h + 1) * D]
                        r0, c0 = (h % 2) * D, (h // 2) * D
                        nc.tensor.matmul(kv_acc[r0:r0 + D, c0:c0 + D], kh, vh,
                                         start=False, stop=(sc == NSC - 1),
                                         skip_group_check=True)
                        nc.tensor.matmul(ks_acc[r0:r0 + D, h // 2:h // 2 + 1],
                                         kh, ones_col[:],
                                         start=False, stop=(sc == NSC - 1),
                                         skip_group_check=True)
                # evict KV / ksum into blockdiag bf16 layouts
                nc.vector.memset(kv2_sb[:], 0.0)
                nc.vector.memset(ksum2_sb[:], 0.0)
                for h in range(H):
                    c, j = h // 2, h % 2
                    nc.vector.tensor_copy(
                        kv2_sb[j * D:(j + 1) * D, c, j * D:(j + 1) * D],
                        kv_acc[j * D:(j + 1) * D, c * D:(c + 1) * D])
                    nc.vector.tensor_copy(
                        ksum2_sb[j * D:(j + 1) * D, c, h:h + 1],
                        ks_acc[j * D:(j + 1) * D, c:c + 1])
                if dbg:
                    nc.gpsimd.dma_start(dbg_kv2[:], kv2_sb[:])
                    nc.gpsimd.dma_start(dbg_ks2[:], ksum2_sb[:])

        # ===== Phase D: Q, z, attention, out_proj, GN2 stats =====
        sumattn_p = stats.tile([P, MB, NT], F32)
        sq_p = stats.tile([P, FB, NT], F32)
        if rank >= 2:
            with tc.tile_pool(name="qps", bufs=1, space="PSUM") as qps, \
                 tc.tile_pool(name="zps", bufs=1, space="PSUM") as zps, \
                 tc.tile_pool(name="atps", bufs=2, space="PSUM") as atps, \
                 tc.tile_pool(name="yps", bufs=2, space="PSUM") as yps, \
                 tc.tile_pool(name="zbps", bufs=2, space="PSUM") as zbps, \
                 tc.tile_pool(name="delu", bufs=3) as delu, \
                 tc.tile_pool(name="qk", bufs=2) as qkp, \
                 tc.tile_pool(name="asb", bufs=2) as asbp, \
                 tc.tile_pool(name="zbp", bufs=2) as zbp, \
                 tc.tile_pool(name="zrp", bufs=2) as zrp:
                for t in range(NT):
                    s0 = t * ST
                    qk = qkp.tile([P, MB, ST], BF16)
                    z = zps.tile([8, ST], F32)
                    for c in range(MB):
                        qp = qps.tile([P, ST], F32)
                        for fb in range(FB):
                            nc.tensor.matmul(qp[:], wq_sb[:, fb, c * P:(c + 1) * P],
                                             x_slab[:, fb, s0:s0 + ST],
                                             start=(fb == 0), stop=(fb == FB - 1))
                        tmin = delu.tile([P, ST], BF16, tag="tmin")
                        if has_q_bias:
                            nc.vector.tensor_scalar(out=tmin[:], in0=qp[:],
                                                    scalar1=bq_sb[:, c:c + 1],
                                                    scalar2=0.0,
                                                    op0=OP.add, op1=OP.min)
                        else:
                            nc.vector.tensor_scalar_min(tmin[:], qp[:], 0.0)
                        e = delu.tile([P, ST], BF16, tag="e")
                        nc.scalar.activation(e[:], tmin[:], AF.Exp)
                        if has_q_bias:
                            qb = delu.tile([P, ST], F32, tag="qb")
                            nc.vector.tensor_scalar(out=qb[:], in0=qp[:],
                                                    scalar1=bq_sb[:, c:c + 1],
                                                    scalar2=0.0,
                                                    op0=OP.add, op1=OP.max)
                            nc.vector.tensor_tensor(qk[:, c, :], qb[:], e[:],
                                                    op=OP.add)
                        else:
                            nc.vector.scalar_tensor_tensor(out=qk[:, c, :],
                                                           in0=qp[:], scalar=0.0,
                                                           in1=e[:],
                                                           op0=OP.max, op1=OP.add)
                        if dsub >= 2:
                            nc.tensor.matmul(z[:], ksum2_sb[:, c, :], qk[:, c, :],
                                             start=(c == 0), stop=(c == MB - 1),
                                             skip_group_check=True)
                    if dsub < 3:
                        continue
                    zrec = zrp.tile([8, ST], F32, tag="zrec")
                    zscr = zrp.tile([8, ST], F32, tag="zscr")
                    nc.vector.reciprocal_approx_accurate(out=zrec[:], in_=z[:],
                                                         scratch=zscr[:])
                    if dbg:
                        nc.gpsimd.dma_start(dbg_zr[t, :, :], zrec[:])
                    zr_hi = zrp.tile([8, ST], BF16, tag="zr_hi")
                    nc.vector.tensor_copy(zr_hi[:], zrec[:])
                    zr_lo = zrp.tile([8, ST], BF16, tag="zr_lo")
                    nc.vector.scalar_tensor_tensor(out=zr_lo[:], in0=zrec[:],
                                                   scalar=0.0, in1=zr_hi[:],
                                                   op0=OP.add, op1=OP.subtract)
                    if dsub < 4:
                        continue
                    a_sb = asbp.tile([P, MB, ST], BF16)
                    for c in range(MB):
                        zb_ps = zbps.tile([P, ST], F32)
                        nc.tensor.matmul(zb_ps[:], selzb_sb[:, c * P:(c + 1) * P],
                                         zr_hi[:], start=True, stop=False)
                        nc.tensor.matmul(zb_ps[:], selzb_sb[:, c * P:(c + 1) * P],
                                         zr_lo[:], start=False, stop=True)
                        zb = zbp.tile([P, ST], F32)
                        nc.scalar.activation(zb[:], zb_ps[:], AF.Copy)
                        if dsub < 5:
                            continue
                        at = atps.tile([P, ST], F32)
                        nc.tensor.matmul(at[:], kv2_sb[:, c, :], qk[:, c, :],
                                         start=True, stop=True)
                        if dsub == 5:
                            nc.vector.tensor_copy(a_sb[:, c, :], at[:])
                        else:
                            nc.vector.scalar_tensor_tensor(
                                out=a_sb[:, c, :], in0=at[:], scalar=0.0,
                                in1=zb[:], op0=OP.add, op1=OP.mult,
                                accum_out=sumattn_p[:, c, t:t + 1])
                    if dsub < 7:
                        continue
                    for fc in range(FB):
                        yp = yps.tile([P, ST], F32)
                        for c in range(MB):
                            nc.tensor.matmul(yp[:],
                                             pt_sb[:, c, fc * P:(fc + 1) * P],
                                             a_sb[:, c, :],
                                             start=(c == 0), stop=(c == MB - 1))
                        sqs = delu.tile([P, ST], BF16, tag="sqs")
                        nc.scalar.activation(sqs[:], yp[:], AF.Square,
                                             accum_out=sq_p[:, fc, t:t + 1])
                        nc.scalar.activation(y_slab[:, fc, s0:s0 + ST], yp[:],
                                             AF.Copy)

        # =========== Phase E: finalize GN2 + gate ===========
        if rank >= 3:
            sa = stats.tile([P, MB], F32)
            nc.vector.tensor_reduce(sa[:], sumattn_p[:],
                                    axis=mybir.AxisListType.X, op=OP.add)
            sa_bf = stats.tile([P, MB], BF16)
            nc.vector.tensor_copy(sa_bf[:], sa[:])
            with tc.tile_pool(name="psE", bufs=2, space="PSUM") as psE:
                s1y_ps = psE.tile([P, FB], F32, tag="ps_small")
                nc.vector.memset(s1y_ps[:], 0.0)
                for fc in range(FB):
                    for c in range(MB):
                        nc.tensor.matmul(s1y_ps[:, fc:fc + 1],
                                         pt_sb[:, c, fc * P:(fc + 1) * P],
                                         sa_bf[:, c:c + 1],
                                         start=False, stop=(c == MB - 1),
                                         skip_group_check=True)
                mean_y = stats.tile([P, FB], F32)
                nc.vector.tensor_scalar_mul(mean_y[:], s1y_ps[:], 1.0 / S)
                s2y = stats.tile([P, FB], F32)
                nc.vector.tensor_reduce(s2y[:], sq_p[:],
                                        axis=mybir.AxisListType.X, op=OP.add)
                e2_y = stats.tile([P, FB], F32)
                nc.vector.tensor_scalar_mul(e2_y[:], s2y[:], 1.0 / S)
                scale2, bias2 = groupnorm_finalize(mean_y[:], e2_y[:], g2_sb[:],
                                                   b2_sb[:], small, psE)
            scale2g = stats.tile([P, FB], F32)
            nc.vector.tensor_tensor(scale2g[:], scale2[:], gate_sb[:], op=OP.mult)
            bias2g = stats.tile([P, FB], F32)
            nc.vector.tensor_tensor(bias2g[:], bias2[:], gate_sb[:], op=OP.mult)
            if dbg:
                nc.gpsimd.dma_start(dbg_xn[:], x_slab[:])
                nc.gpsimd.dma_start(dbg_y[:], y_slab[:])
                nc.gpsimd.dma_start(dbg_sb2[:, :, 0], scale2[:])
                nc.gpsimd.dma_start(dbg_sb2[:, :, 1], bias2[:])
                nc.gpsimd.dma_start(dbg_sb2[:, :, 2], scale1[:])
                nc.gpsimd.dma_start(dbg_sb2[:, :, 3], bias1[:])

        # =========== Phase F: residual + store ===========
        if rank >= 4:
            with tc.tile_pool(name="fstage", bufs=2) as fstage, \
                 tc.tile_pool(name="fysc", bufs=2) as fysc, \
                 tc.tile_pool(name="fout", bufs=2) as foutp:
                for t in range(NT):
                    s0 = t * ST
                    hst = fstage.tile([P, FB, ST], F32)
                    nc.gpsimd.dma_start(hst[:], hidden_v[:, :, s0:s0 + ST])
                    ysc = fysc.tile([P, FB, ST], BF16)
                    fo = foutp.tile([P, FB, ST], F32)
                    for fb in range(FB):
                        nc.gpsimd.tensor_scalar(out=ysc[:, fb, :],
                                                in0=y_slab[:, fb, s0:s0 + ST],
                                                scalar1=scale2g[:, fb:fb + 1],
                                                scalar2=bias2g[:, fb:fb + 1],
                                                op0=OP.mult, op1=OP.add)
                        nc.gpsimd.tensor_tensor(fo[:, fb, :], hst[:, fb, :],
                                                ysc[:, fb, :], op=OP.add)
                    nc.gpsimd.dma_start(out_v[:, :, s0:s0 + ST], fo[:])

    nc.finalize()
    return nc


_PROGRAM_CACHE: dict = {}


def _get_program(has_q_bias: bool, has_kv_bias: bool):
    key = (has_q_bias, has_kv_bias)
    if key not in _PROGRAM_CACHE:
        _PROGRAM_CACHE[key] = _build_program(has_q_bias, has_kv_bias)
    return _PROGRAM_CACHE[key]


def _host_inputs(hidden_b, qkv_w, qkv_b, out_proj, gn1_gamma, gn1_beta,
                 gn2_gamma, gn2_beta, gate_g, has_q_bias, has_kv_bias):
    """Build the per-core input map (hidden_b is this core's [F, S] slice)."""
    bf = ml_dtypes.bfloat16
    w = np.asarray(qkv_w, np.float32).reshape(3, F, F)  # [3, m=(h,d), f]
    wq_t = np.ascontiguousarray(w[0].T).astype(bf)       # [f, m]
    wkv_t = np.ascontiguousarray(
        np.concatenate([w[1].T, w[2].T], axis=1)).astype(bf)  # [f, 2F]
    p_t = np.ascontiguousarray(np.asarray(out_proj, np.float32).T).astype(bf)

    pg = np.arange(P) // 16
    sel_g = np.zeros((P, 8), np.float32)
    sel_g[np.arange(P), pg] = 1.0
    sel_b = np.ascontiguousarray(sel_g.T)
    sel_zb = np.zeros((8, MB * P), np.float32)
    for c in range(MB):
        for p in range(P):
            sel_zb[2 * c + p // D, c * P + p] = 1.0
    sel_zb = sel_zb.astype(bf)

    def chan(v):  # [F] -> [P, FB] with c = fb*128 + p
        return np.ascontiguousarray(
            np.asarray(v, np.float32).reshape(FB, P).T)

    m = {
        "hidden": np.ascontiguousarray(hidden_b.astype(np.float32)),
        "wq_t": wq_t, "wkv_t": wkv_t, "p_t": p_t,
        "sel_g": sel_g, "sel_b": sel_b, "sel_zb": sel_zb,
        "gamma1c": chan(gn1_gamma), "beta1c": chan(gn1_beta),
        "gamma2c": chan(gn2_gamma), "beta2c": chan(gn2_beta),
        "gatec": chan(np.asarray(gate_g, np.float32).reshape(F)),
    }
    b = np.asarray(qkv_b, np.float32).reshape(3, F)
    if has_q_bias:
        m["bq"] = chan(b[0])
    if has_kv_bias:
        m["bkv"] = np.ascontiguousarray(
            np.concatenate([b[1], b[2]])[None, :]).astype(bf)
    return m


def kernel(hidden_states, qkv_w, qkv_b, out_proj, gn1_gamma, gn1_beta,
           gn2_gamma, gn2_beta, gate_g, _trace=False, _tmpdir=None):
    hidden_states = np.asarray(hidden_states, np.float32)
    b = np.asarray(qkv_b, np.float32).reshape(3, F)
    has_q_bias = bool(np.any(b[0] != 0.0))
    has_kv_bias = bool(np.any(b[1:] != 0.0))
    nc = _get_program(has_q_bias, has_kv_bias)

    in_maps = []
    for core in range(N_CORES):
        in_maps.append(_host_inputs(hidden_states[core], qkv_w, qkv_b, out_proj,
                                    gn1_gamma, gn1_beta, gn2_gamma, gn2_beta,
                                    gate_g, has_q_bias, has_kv_bias))
    res = run_bass_kernel_spmd(nc, in_maps, core_ids=list(range(N_CORES)),
                               trace=_trace, tmpdir=_tmpdir)
    outs = np.stack([np.asarray(res.results[c]["out"], np.float32)
                     for c in range(N_CORES)], axis=0)
    kernel._last_results = res
    return outs
